# revision 5
# baseline (speedup 1.0000x reference)
"""Trainium2 Bass kernel for DepthWiseSeparableAttention.

Math notes (all exact identities, no approximations):
- The depthwise-conv "local bias" in the reference is constant along the
  softmax axis, so it cancels in softmax and is skipped entirely.
- Eval-mode BatchNorm, the LayerNorm affine (gamma/beta) and the attention
  scale fold into the qkv weight/bias on the host.
- K's effective bias adds a per-query constant to scores -> cancels in
  softmax -> dropped.  V's effective bias shifts attention output by a
  constant vector (softmax rows sum to 1) -> folded through proj_w into
  proj_b, which is folded into the residual input.
- Softmax denominators come from a ones-column appended to V (the PV matmul
  then computes per-query colsums for free); normalization is applied at
  O-eviction time with a DMA partition-broadcast of the reciprocal row.

Distribution: data-parallel over the batch dim - 8 batch elements, one per
NeuronCore, identical SPMD program, no collectives.
"""

import numpy as np

B, N, C = 8, 1024, 512
HEADS, DH = 8, 64
SCALE = DH ** -0.5
NT = N // 128   # 8 token chunks
CT = C // 128   # 4 channel chunks

P_BF16 = True   # probabilities/V in bf16 (else float32r)

_CACHE = {}


def _build_program(p_bf16):
    from contextlib import ExitStack

    import concourse.bacc as bacc
    import concourse.tile as tile
    from concourse import mybir

    from concourse.bass import ts

    f32 = mybir.dt.float32
    f32r = mybir.dt.float32r
    bf16 = mybir.dt.bfloat16
    p_dt = bf16 if p_bf16 else f32r
    Act = mybir.ActivationFunctionType
    Alu = mybir.AluOpType

    nc = bacc.Bacc(None, target_bir_lowering=False)

    x_d = nc.declare_dram_parameter("x", [N, C], f32, isOutput=False)
    xr_d = nc.declare_dram_parameter("xr", [N, C], f32, isOutput=False)
    wqk_d = nc.declare_dram_parameter("wqk", [C, 2 * C], f32r, isOutput=False)
    wv_d = nc.declare_dram_parameter("wv", [C, C], f32r, isOutput=False)
    pwt_d = nc.declare_dram_parameter("pwt", [C, C], f32r, isOutput=False)
    bq_d = nc.declare_dram_parameter("bq", [C], f32, isOutput=False)
    iden_d = nc.declare_dram_parameter("iden", [128, 128], f32r, isOutput=False)
    out_d = nc.declare_dram_parameter("out", [N, C], f32, isOutput=True)

    with tile.TileContext(nc) as tc, ExitStack() as stk:
        const = stk.enter_context(tc.tile_pool(name="const", bufs=1))
        big = stk.enter_context(tc.tile_pool(name="big", bufs=1))

        wqk_sb = const.tile([128, CT, 2 * C], f32r)   # [p, cc, o]
        wv_sb = const.tile([128, CT, C], f32r)
        pwt_sb = const.tile([128, CT, C], f32r)
        bq_sb = const.tile([128, CT], f32)
        iden = const.tile([128, 128], f32r)
        eps = const.tile([128, 1], f32)
        nc.sync.dma_start(out=wqk_sb[:], in_=wqk_d.rearrange("(cc p) o -> p cc o", p=128))
        nc.sync.dma_start(out=wv_sb[:], in_=wv_d.rearrange("(cc p) o -> p cc o", p=128))
        nc.sync.dma_start(out=pwt_sb[:], in_=pwt_d.rearrange("(cc p) o -> p cc o", p=128))
        nc.sync.dma_start(out=bq_sb[:], in_=bq_d.rearrange("(cc p) -> p cc", p=128))
        nc.sync.dma_start(out=iden[:], in_=iden_d[:])
        nc.vector.memset(eps[:], 1e-6)

        xnT = big.tile([128, CT, N], f32r)        # xn^T: [c_local, cc, tokens]
        qkT = big.tile([128, 2 * CT, N], f32r)    # qkv^T q|k: [o_local, oc, tokens]
        v_sb = big.tile([128, NT, HEADS, DH + 1], p_dt)  # V natural + ones col
        ot = big.tile([128, CT, N], f32r)         # normalized O^T

        nc.vector.memset(v_sb[:, :, :, DH:DH + 1], 1.0)

        # ---- Phase 1: LayerNorm + transpose to xnT -------------------------
        with (
            tc.tile_pool(name="px", bufs=3) as px,
            tc.tile_pool(name="pstat", bufs=4) as pstat,
            tc.tile_pool(name="psA", bufs=2, space="PSUM") as psA,
        ):
            for tcn in range(NT):
                x_sb = px.tile([128, C], f32, tag="x")
                nc.sync.dma_start(out=x_sb[:], in_=x_d[ts(tcn, 128), :])
                stats = pstat.tile([128, 6], f32, tag="st")
                nc.vector.bn_stats(out=stats[:], in_=x_sb[:])
                mv = pstat.tile([128, 2], f32, tag="mv")
                nc.vector.bn_aggr(out=mv[:], in_=stats[:])
                rstd = pstat.tile([128, 1], f32, tag="rstd")
                nc.scalar.activation(out=rstd[:], in_=mv[:, 1:2], func=Act.Sqrt,
                                     bias=eps[:], scale=1.0)
                nc.vector.reciprocal(out=rstd[:], in_=rstd[:])
                xn = px.tile([128, C], f32r, tag="xn")
                nc.vector.tensor_scalar(out=xn[:], in0=x_sb[:],
                                        scalar1=mv[:, 0:1], scalar2=rstd[:],
                                        op0=Alu.subtract, op1=Alu.mult)
                pt = psA.tile([128, 512], f32r, tag="pt")
                for cc in range(CT):
                    nc.tensor.transpose(pt[:, ts(cc, 128)], xn[:, ts(cc, 128)], iden[:])
                nc.vector.tensor_copy(
                    out=xnT[:, :, ts(tcn, 128)],
                    in_=pt[:].rearrange("p (cc t) -> p cc t", cc=CT),
                )

            # ---- Phase 2: q/k projection (transposed layout) ---------------
            for oc in range(2 * CT):
                for nt in range(2):
                    qk_ps = psA.tile([128, 512], f32, tag="qk")
                    for cc in range(CT):
                        nc.tensor.matmul(
                            qk_ps[:],
                            wqk_sb[:, cc, ts(oc, 128)],
                            xnT[:, cc, ts(nt, 512)],
                            start=(cc == 0), stop=(cc == CT - 1),
                        )
                    if oc < CT:  # q: add bias (k bias cancels in softmax)
                        nc.vector.tensor_scalar(
                            out=qkT[:, oc, ts(nt, 512)], in0=qk_ps[:],
                            scalar1=bq_sb[:, oc:oc + 1], scalar2=None, op0=Alu.add)
                    else:
                        nc.vector.tensor_copy(out=qkT[:, oc, ts(nt, 512)], in_=qk_ps[:])

            # ---- Phase 3: v projection (natural layout) --------------------
            for tcn in range(NT):
                v_ps = psA.tile([128, 512], f32, tag="v")
                for cc in range(CT):
                    nc.tensor.matmul(
                        v_ps[:],
                        xnT[:, cc, ts(tcn, 128)],
                        wv_sb[:, cc, :],
                        start=(cc == 0), stop=(cc == CT - 1),
                    )
                nc.vector.tensor_copy(
                    out=v_sb[:, tcn, :, 0:DH],
                    in_=v_ps[:].rearrange("p (h d) -> p h d", h=HEADS),
                )

        # ---- Phase 4: attention, head pairs --------------------------------
        with (
            tc.tile_pool(name="pp", bufs=3) as pp,
            tc.tile_pool(name="pr", bufs=4) as pr,
            tc.tile_pool(name="prd", bufs=4, space="DRAM") as prd,
            tc.tile_pool(name="psS", bufs=2, space="PSUM") as psS,
            tc.tile_pool(name="psO", bufs=2, space="PSUM") as psO,
        ):
            for hp in range(4):
                qc, kc_ = hp, CT + hp
                p0 = pp.tile([128, NT, N], p_dt, tag="p")
                p1 = pp.tile([128, NT, N], p_dt, tag="p")
                for kc in range(NT):
                    s0 = psS.tile([128, N], f32, tag="s")
                    s1 = psS.tile([128, N], f32, tag="s")
                    for nt2 in range(2):
                        nc.tensor.matmul(
                            s0[:, ts(nt2, 512)],
                            qkT[0:64, kc_, ts(kc, 128)],
                            qkT[0:64, qc, ts(nt2, 512)],
                        )
                        nc.tensor.matmul(
                            s1[:, ts(nt2, 512)],
                            qkT[64:128, kc_, ts(kc, 128)],
                            qkT[64:128, qc, ts(nt2, 512)],
                        )
                    nc.scalar.activation(out=p0[:, kc, :], in_=s0[:], func=Act.Exp)
                    nc.scalar.activation(out=p1[:, kc, :], in_=s1[:], func=Act.Exp)

                o_ps0 = psO.tile([DH + 1, N], f32, tag="o")
                o_ps1 = psO.tile([DH + 1, N], f32, tag="o")
                for kc in range(NT):
                    for nt2 in range(2):
                        nc.tensor.matmul(
                            o_ps0[:, ts(nt2, 512)],
                            v_sb[:, kc, 2 * hp, :],
                            p0[:, kc, ts(nt2, 512)],
                            start=(kc == 0), stop=(kc == NT - 1),
                        )
                        nc.tensor.matmul(
                            o_ps1[:, ts(nt2, 512)],
                            v_sb[:, kc, 2 * hp + 1, :],
                            p1[:, kc, ts(nt2, 512)],
                            start=(kc == 0), stop=(kc == NT - 1),
                        )
                for hsub, o_ps in ((0, o_ps0), (1, o_ps1)):
                    # reciprocal stays on partition 64 (DVE lanes are vertical);
                    # bounce through DRAM, then a broadcast-read fans it across
                    # partitions 0..63 (SBUF APs forbid zero partition step).
                    recip = pr.tile([DH + 1, N], f32, tag="recip")
                    nc.vector.reciprocal(out=recip[DH:DH + 1, :], in_=o_ps[DH:DH + 1, :])
                    rd = prd.tile([1, N], f32, tag="rd")
                    nc.sync.dma_start(out=rd[:], in_=recip[DH:DH + 1, :])
                    rb = pr.tile([64, N], f32, tag="rb")
                    nc.sync.dma_start(out=rb[:], in_=rd[:].to_broadcast((64, N)))
                    if hsub == 0:
                        nc.vector.tensor_mul(out=ot[0:64, hp, :],
                                             in0=o_ps[0:DH, :], in1=rb[:])
                    else:
                        o_tmp = pr.tile([64, N], f32r, tag="otmp")
                        nc.vector.tensor_mul(out=o_tmp[:],
                                             in0=o_ps[0:DH, :], in1=rb[:])
                        nc.sync.dma_start(out=ot[64:128, hp, :], in_=o_tmp[:])

        # ---- Phase 5: output projection + residual -------------------------
        with (
            tc.tile_pool(name="pout", bufs=3) as pout,
            tc.tile_pool(name="psY", bufs=4, space="PSUM") as psY,
        ):
            for tcn in range(NT):
                y_ps = psY.tile([128, 512], f32, tag="y")
                for cc in range(CT):
                    nc.tensor.matmul(
                        y_ps[:],
                        ot[:, cc, ts(tcn, 128)],
                        pwt_sb[:, cc, :],
                        start=(cc == 0), stop=(cc == CT - 1),
                    )
                xr_sb = pout.tile([128, C], f32, tag="xr")
                nc.sync.dma_start(out=xr_sb[:], in_=xr_d[ts(tcn, 128), :])
                y_sb = pout.tile([128, C], f32, tag="y")
                nc.vector.tensor_add(out=y_sb[:], in0=y_ps[:], in1=xr_sb[:])
                nc.sync.dma_start(out=out_d[ts(tcn, 128), :], in_=y_sb[:])

    nc.compile()
    return nc


def _prepare_host(inputs):
    f64 = np.float64
    x = np.asarray(inputs["x"], np.float32)
    qkv_w = np.asarray(inputs["qkv_w"], f64)
    qkv_b = np.asarray(inputs["qkv_b"], f64)
    g = np.asarray(inputs["ln_gamma"], f64)
    beta = np.asarray(inputs["ln_beta"], f64)
    s_bn = np.asarray(inputs["bn_gamma"], f64) / np.sqrt(
        np.asarray(inputs["bn_var"], f64) + 1e-5)
    bn_beta = np.asarray(inputs["bn_beta"], f64)
    bn_mean = np.asarray(inputs["bn_mean"], f64)
    proj_w = np.asarray(inputs["proj_w"], f64)
    proj_b = np.asarray(inputs["proj_b"], f64)

    w_eff = qkv_w * s_bn[:, None] * g[None, :]
    b_full = s_bn * (qkv_w @ beta + qkv_b - bn_mean) + bn_beta
    w_eff[0:C] *= SCALE
    b_full[0:C] *= SCALE

    wqk = np.ascontiguousarray(w_eff[0:2 * C].T, np.float32)   # [C, 2C]
    wv = np.ascontiguousarray(w_eff[2 * C:3 * C].T, np.float32)  # [C, C]
    pwt = np.ascontiguousarray(proj_w.T, np.float32)            # [C, C]
    bq = b_full[0:C].astype(np.float32)
    pb_eff = (proj_b + proj_w @ b_full[2 * C:3 * C]).astype(np.float32)
    xr = (x + pb_eff[None, None, :]).astype(np.float32)
    iden = np.eye(128, dtype=np.float32)
    return x, xr, wqk, wv, pwt, bq, iden


def kernel(**inputs):
    from concourse.bass_utils import run_bass_kernel_spmd

    x, xr, wqk, wv, pwt, bq, iden = _prepare_host(inputs)

    if "nc" not in _CACHE:
        _CACHE["nc"] = _build_program(P_BF16)
    nc = _CACHE["nc"]

    in_maps = [
        {"x": np.ascontiguousarray(x[b]), "xr": np.ascontiguousarray(xr[b]),
         "wqk": wqk, "wv": wv, "pwt": pwt, "bq": bq, "iden": iden}
        for b in range(B)
    ]
    res = run_bass_kernel_spmd(nc, in_maps, list(range(B)))
    _CACHE["last_result"] = res
    out = np.stack([res.results[b]["out"] for b in range(B)])
    return out.astype(np.float32)
